# revision 1
# baseline (speedup 1.0000x reference)
"""GNN message passing (2-layer, residual) on 8 TRN2 NeuronCores.

Strategy: shard destination nodes across 8 cores (12500 rows each, 98
blocks of 128). Host sorts edges by (dest block, src), pads each block
to T slices of 128 edges. Device gathers neighbor rows by src index
(indirect DMA), scatter-adds them into the dest block via a one-hot
matmul accumulated in PSUM (aggT = G.T @ M), then applies the per-layer
linear/relu. Two launches: layer 0 produces h shards, host concats the
full h (halo exchange), launch 2 does layer 1 + residual + projection.
"""
import os
import sys
import types
import contextlib
import ctypes

import numpy as np

import concourse.bass as bass
import concourse.tile as tile
from concourse import bacc, mybir
from concourse.bass_utils import run_bass_kernel_spmd

N = 100000
E = 640000
D = 128
NC = 8
R = N // NC          # 12500 rows per core
NB = (R + 127) // 128  # 98 blocks; last block has 84 rows
P = 128

PROFILE = bool(int(os.environ.get("GNN_PROFILE", "0")))
LAST_EXEC_NS = []    # per-launch exec_time_ns when PROFILE


def _install_ntff_shim():
    if "antenv.axon_hooks" in sys.modules:
        return
    mod = types.ModuleType("antenv.axon_hooks")
    mod._hook = None
    mod.set_axon_ntff_profile_hook = lambda h: setattr(mod, "_hook", h)
    mod.get_axon_ntff_profile_hook = lambda: mod._hook
    sys.modules["antenv.axon_hooks"] = mod
    try:
        import antenv
        antenv.axon_hooks = mod
        from trn_agent_boot.trn_boot import _ntff_profile_via_ctypes
        mod.set_axon_ntff_profile_hook(
            _ntff_profile_via_ctypes("/opt/axon/libaxon_pjrt.so"))
    except Exception:
        pass


def _prep_edges(edge_index):
    """Per-core padded slice schedule. Per-block slice count T_b is the max
    over cores (SPMD: one program for all cores). Returns colsT [NC,128,S]
    i32, rlT [NC,128,S] f32 (128.0 = padding sentinel), T_arr [NB], offs
    [NB] (slice start per block)."""
    row = edge_index[0].astype(np.int64)
    col = edge_index[1].astype(np.int64)
    per_core = []
    tmax = np.zeros(NB, dtype=np.int64)
    for k in range(NC):
        m = (row // R) == k
        r_loc = (row[m] - k * R).astype(np.int64)
        c = col[m].astype(np.int32)
        blk = r_loc >> 7
        rl = (r_loc & 127).astype(np.int32)
        order = np.lexsort((c, blk))
        blk, rl, c = blk[order], rl[order], c[order]
        counts = np.bincount(blk, minlength=NB)
        tmax = np.maximum(tmax, (counts + P - 1) // P)
        per_core.append((blk, rl, c, counts))
    T_arr = np.maximum(tmax, 1)
    offs = np.zeros(NB, dtype=np.int64)
    offs[1:] = np.cumsum(T_arr)[:-1]
    S = int(T_arr.sum())
    colsT = np.zeros((NC, P, S), dtype=np.int32)
    rlT = np.full((NC, P, S), 128.0, dtype=np.float32)
    for k in range(NC):
        blk, rl, c, counts = per_core[k]
        starts = np.zeros(NB, dtype=np.int64)
        starts[1:] = np.cumsum(counts)[:-1]
        pos = np.arange(len(blk)) - starts[blk]
        s = offs[blk] + pos // P
        p = pos % P
        colsT[k][p, s] = c
        rlT[k][p, s] = rl.astype(np.float32)
    return colsT, rlT, T_arr, offs


def _build_layer0(T_arr, offs):
    S = int(T_arr.sum())
    nc = bacc.Bacc("TRN2", target_bir_lowering=False, debug=False,
                   num_devices=NC)
    x_d = nc.dram_tensor("x", [N, D], mybir.dt.float32, kind="ExternalInput")
    cols_d = nc.dram_tensor("cols", [P, S], mybir.dt.int32, kind="ExternalInput")
    rl_d = nc.dram_tensor("rl", [P, S], mybir.dt.float32, kind="ExternalInput")
    w0_d = nc.dram_tensor("w0", [D, D], mybir.dt.float32, kind="ExternalInput")
    b0_d = nc.dram_tensor("b0", [1, D], mybir.dt.float32, kind="ExternalInput")
    h_d = nc.dram_tensor("h", [R, D], mybir.dt.float32, kind="ExternalOutput")

    with tile.TileContext(nc) as tc:
        with contextlib.ExitStack() as ctx:
            const = ctx.enter_context(tc.tile_pool(name="const", bufs=1))
            gp = ctx.enter_context(tc.tile_pool(name="gp", bufs=6))
            mp = ctx.enter_context(tc.tile_pool(name="mp", bufs=6))
            sp = ctx.enter_context(tc.tile_pool(name="sp", bufs=3))
            hp = ctx.enter_context(tc.tile_pool(name="hp", bufs=3))
            pa = ctx.enter_context(tc.tile_pool(name="pa", bufs=2, space="PSUM"))
            ph = ctx.enter_context(tc.tile_pool(name="ph", bufs=2, space="PSUM"))

            colsSB = const.tile([P, S], mybir.dt.int32)
            rlSB = const.tile([P, S], mybir.dt.float32)
            nc.sync.dma_start(out=colsSB[:], in_=cols_d[:])
            nc.sync.dma_start(out=rlSB[:], in_=rl_d[:])
            w0SB = const.tile([D, D], mybir.dt.float32)
            b0SB = const.tile([1, D], mybir.dt.float32)
            nc.sync.dma_start(out=w0SB[:], in_=w0_d[:])
            nc.sync.dma_start(out=b0SB[:], in_=b0_d[:])
            ones1 = const.tile([1, P], mybir.dt.float32)
            nc.vector.memset(ones1[:], 1.0)
            iotaI = const.tile([P, P], mybir.dt.int32)
            nc.gpsimd.iota(iotaI[:], pattern=[[1, P]], base=0,
                           channel_multiplier=0)
            iotaF = const.tile([P, P], mybir.dt.float32)
            nc.vector.tensor_copy(iotaF[:], iotaI[:])

            for b in range(NB):
                rows_b = min(P, R - b * P)
                T_b = int(T_arr[b])
                psumA = pa.tile([P, P], mybir.dt.float32, tag="pa")
                for j in range(T_b):
                    s = int(offs[b]) + j
                    gb = gp.tile([P, P], mybir.dt.float32, tag="g")
                    nc.gpsimd.indirect_dma_start(
                        out=gb[:], out_offset=None, in_=x_d[:],
                        in_offset=bass.IndirectOffsetOnAxis(
                            ap=colsSB[:, s:s + 1], axis=0))
                    M = mp.tile([P, P], mybir.dt.float32, tag="m")
                    nc.vector.tensor_scalar(
                        out=M[:], in0=iotaF[:], scalar1=rlSB[:, s:s + 1],
                        scalar2=None, op0=mybir.AluOpType.is_equal)
                    nc.tensor.matmul(psumA[:], lhsT=gb[:], rhs=M[:],
                                     start=(j == 0), stop=(j == T_b - 1))
                sA = sp.tile([P, P], mybir.dt.float32, tag="sa")
                nc.vector.tensor_copy(sA[:], psumA[:])
                psumH = ph.tile([P, P], mybir.dt.float32, tag="phh")
                nc.tensor.matmul(psumH[:], lhsT=sA[:], rhs=w0SB[:],
                                 start=True, stop=False)
                nc.tensor.matmul(psumH[:], lhsT=ones1[:], rhs=b0SB[:],
                                 start=False, stop=True)
                hsb = hp.tile([P, P], mybir.dt.float32, tag="h")
                nc.scalar.activation(hsb[:], psumH[:],
                                     mybir.ActivationFunctionType.Relu)
                nc.sync.dma_start(out=h_d[b * P:b * P + rows_b, :],
                                  in_=hsb[:rows_b, :])
    nc.compile()
    return nc


def _build_layer1(T_arr, offs):
    S = int(T_arr.sum())
    nc = bacc.Bacc("TRN2", target_bir_lowering=False, debug=False,
                   num_devices=NC)
    hf_d = nc.dram_tensor("hf", [N, D], mybir.dt.float32, kind="ExternalInput")
    cols_d = nc.dram_tensor("cols", [P, S], mybir.dt.int32, kind="ExternalInput")
    rl_d = nc.dram_tensor("rl", [P, S], mybir.dt.float32, kind="ExternalInput")
    w1_d = nc.dram_tensor("w1", [D, D], mybir.dt.float32, kind="ExternalInput")
    b1_d = nc.dram_tensor("b1", [P, 1], mybir.dt.float32, kind="ExternalInput")
    wp_d = nc.dram_tensor("wp", [D, D], mybir.dt.float32, kind="ExternalInput")
    bp_d = nc.dram_tensor("bp", [1, D], mybir.dt.float32, kind="ExternalInput")
    o_d = nc.dram_tensor("o", [R, D], mybir.dt.float32, kind="ExternalOutput")

    with tile.TileContext(nc) as tc:
        with contextlib.ExitStack() as ctx:
            const = ctx.enter_context(tc.tile_pool(name="const", bufs=1))
            gp = ctx.enter_context(tc.tile_pool(name="gp", bufs=6))
            mp = ctx.enter_context(tc.tile_pool(name="mp", bufs=6))
            sp = ctx.enter_context(tc.tile_pool(name="sp", bufs=3))
            hp = ctx.enter_context(tc.tile_pool(name="hp", bufs=3))
            pa = ctx.enter_context(tc.tile_pool(name="pa", bufs=2, space="PSUM"))
            pz = ctx.enter_context(tc.tile_pool(name="pz", bufs=2, space="PSUM"))
            po = ctx.enter_context(tc.tile_pool(name="po", bufs=2, space="PSUM"))

            colsSB = const.tile([P, S], mybir.dt.int32)
            rlSB = const.tile([P, S], mybir.dt.float32)
            nc.sync.dma_start(out=colsSB[:], in_=cols_d[:])
            nc.sync.dma_start(out=rlSB[:], in_=rl_d[:])
            w1SB = const.tile([D, D], mybir.dt.float32)
            b1SB = const.tile([P, 1], mybir.dt.float32)
            wpSB = const.tile([D, D], mybir.dt.float32)
            bpSB = const.tile([1, D], mybir.dt.float32)
            nc.sync.dma_start(out=w1SB[:], in_=w1_d[:])
            nc.sync.dma_start(out=b1SB[:], in_=b1_d[:])
            nc.sync.dma_start(out=wpSB[:], in_=wp_d[:])
            nc.sync.dma_start(out=bpSB[:], in_=bp_d[:])
            ones1 = const.tile([1, P], mybir.dt.float32)
            nc.vector.memset(ones1[:], 1.0)
            iotaI = const.tile([P, P], mybir.dt.int32)
            nc.gpsimd.iota(iotaI[:], pattern=[[1, P]], base=0,
                           channel_multiplier=0)
            iotaF = const.tile([P, P], mybir.dt.float32)
            nc.vector.tensor_copy(iotaF[:], iotaI[:])

            for b in range(NB):
                rows_b = min(P, R - b * P)
                T_b = int(T_arr[b])
                psumA = pa.tile([P, P], mybir.dt.float32, tag="pa")
                for j in range(T_b):
                    s = int(offs[b]) + j
                    gb = gp.tile([P, P], mybir.dt.float32, tag="g")
                    nc.gpsimd.indirect_dma_start(
                        out=gb[:], out_offset=None, in_=hf_d[:],
                        in_offset=bass.IndirectOffsetOnAxis(
                            ap=colsSB[:, s:s + 1], axis=0))
                    M = mp.tile([P, P], mybir.dt.float32, tag="m")
                    nc.vector.tensor_scalar(
                        out=M[:], in0=iotaF[:], scalar1=rlSB[:, s:s + 1],
                        scalar2=None, op0=mybir.AluOpType.is_equal)
                    nc.tensor.matmul(psumA[:], lhsT=gb[:], rhs=M[:],
                                     start=(j == 0), stop=(j == T_b - 1))
                sA1 = sp.tile([P, P], mybir.dt.float32, tag="sa")
                nc.vector.tensor_copy(sA1[:], psumA[:])       # agg1T [feat, rows]
                psumZ = pz.tile([P, P], mybir.dt.float32, tag="pz")
                nc.tensor.matmul(psumZ[:], lhsT=w1SB[:], rhs=sA1[:],
                                 start=True, stop=True)        # (agg1@W1).T
                t1 = hp.tile([P, P], mybir.dt.float32, tag="t1")
                nc.scalar.activation(t1[:], psumZ[:],
                                     mybir.ActivationFunctionType.Relu,
                                     bias=b1SB[:])              # relu(zT + b1)
                h2T = hp.tile([P, P], mybir.dt.float32, tag="h2")
                nc.vector.tensor_add(h2T[:], t1[:], sA1[:])     # + agg1 (residual)
                psumO = po.tile([P, P], mybir.dt.float32, tag="po")
                nc.tensor.matmul(psumO[:], lhsT=h2T[:], rhs=wpSB[:],
                                 start=True, stop=False)
                nc.tensor.matmul(psumO[:], lhsT=ones1[:], rhs=bpSB[:],
                                 start=False, stop=True)        # h2@Wp + bp
                osb = hp.tile([P, P], mybir.dt.float32, tag="o")
                nc.vector.tensor_copy(osb[:], psumO[:])
                nc.sync.dma_start(out=o_d[b * P:b * P + rows_b, :],
                                  in_=osb[:rows_b, :])
    nc.compile()
    return nc


def _run(nc, in_maps):
    global LAST_EXEC_NS
    res = run_bass_kernel_spmd(nc, in_maps, core_ids=list(range(NC)),
                               trace=PROFILE)
    if PROFILE:
        LAST_EXEC_NS.append(res.exec_time_ns)
    return res.results


def kernel(x, edge_index, W0, b0, W1, b1, Wp, bp):
    global LAST_EXEC_NS
    LAST_EXEC_NS = []
    if PROFILE:
        _install_ntff_shim()
    x = np.ascontiguousarray(x, dtype=np.float32)
    W0 = np.ascontiguousarray(W0, dtype=np.float32)
    W1 = np.ascontiguousarray(W1, dtype=np.float32)
    Wp = np.ascontiguousarray(Wp, dtype=np.float32)
    colsT, rlT, T_arr, offs = _prep_edges(np.asarray(edge_index))

    nc0 = _build_layer0(T_arr, offs)
    in0 = [{"x": x, "cols": colsT[k], "rl": rlT[k],
            "w0": W0, "b0": np.asarray(b0, np.float32).reshape(1, D)}
           for k in range(NC)]
    res0 = _run(nc0, in0)
    hfull = np.concatenate([res0[k]["h"] for k in range(NC)], axis=0)

    nc1 = _build_layer1(T_arr, offs)
    in1 = [{"hf": hfull, "cols": colsT[k], "rl": rlT[k],
            "w1": W1, "b1": np.asarray(b1, np.float32).reshape(P, 1),
            "wp": Wp, "bp": np.asarray(bp, np.float32).reshape(1, D)}
           for k in range(NC)]
    res1 = _run(nc1, in1)
    out = np.concatenate([res1[k]["o"] for k in range(NC)], axis=0)
    return out



# revision 2
# speedup vs baseline: 1.0015x; 1.0015x over previous
"""GNN message passing (2-layer, residual) on 8 TRN2 NeuronCores — v3.

v5 = v3 (group-padded slices, block-crossing pairs, DVE one-hot
builds) + one-hot masks for half the blocks streamed from HBM
(host-precomputed) to offload the DVE, + gather queue round-robin.
"""
import os
import sys
import types
import contextlib

import numpy as np
import ml_dtypes

import concourse.bass as bass
import concourse.tile as tile
from concourse import bacc, mybir
from concourse.bass_utils import run_bass_kernel_spmd
from concourse.library_config import mlp as mlp_lib

N = 100000
E = 640000
D = 128
NC = 8
R = N // NC            # 12500 rows per core
NB = (R + 127) // 128  # 98 blocks; last block has 84 rows
P = 128
CH = 25000             # gather chunk (int16 idx limit 32767)
NCH = 4
G = 14                 # blocks per group
NG = (NB + G - 1) // NCH if False else (NB + G - 1) // G
MAX_SLICE_PER_GATHER = 8   # 1024 idx cap per dma_gather
SENT = 200.0

BF16 = mybir.dt.bfloat16
bf16 = ml_dtypes.bfloat16

PROFILE = bool(int(os.environ.get("GNN_PROFILE", "0")))
LAST_EXEC_NS = []


def _install_ntff_shim():
    if "antenv.axon_hooks" in sys.modules:
        return
    mod = types.ModuleType("antenv.axon_hooks")
    mod._hook = None
    mod.set_axon_ntff_profile_hook = lambda h: setattr(mod, "_hook", h)
    mod.get_axon_ntff_profile_hook = lambda: mod._hook
    sys.modules["antenv.axon_hooks"] = mod
    try:
        import antenv
        antenv.axon_hooks = mod
        from trn_agent_boot.trn_boot import _ntff_profile_via_ctypes
        mod.set_axon_ntff_profile_hook(
            _ntff_profile_via_ctypes("/opt/axon/libaxon_pjrt.so"))
    except Exception:
        pass


class Sched:
    pass


def _prep_edges(edge_index):
    row = np.asarray(edge_index[0], dtype=np.int64)
    col = np.asarray(edge_index[1], dtype=np.int64)

    # per core, per (g, c): edge arrays sorted by (b, col)
    core_runs = []          # [k][(g,c)] -> (blk, rl, col_loc)
    n_kgc = np.zeros((NC, NG, NCH), dtype=np.int64)
    seg = {}                # (k,g,c,b) -> (lo, hi) positions in run
    for k in range(NC):
        m = (row // R) == k
        r_loc = (row[m] - k * R).astype(np.int64)
        c = col[m].astype(np.int64)
        blk = r_loc >> 7
        rl = (r_loc & 127).astype(np.int64)
        ch = c // CH
        g = blk // G
        order = np.lexsort((c, blk, ch, g))
        blk, rl, c, ch, g = blk[order], rl[order], c[order], ch[order], g[order]
        runs = {}
        for gg in range(NG):
            for cc in range(NCH):
                mm = (g == gg) & (ch == cc)
                bb, rr, ccol = blk[mm], rl[mm], c[mm] - cc * CH
                runs[(gg, cc)] = (bb, rr, ccol)
                n_kgc[k, gg, cc] = len(bb)
                # block segments within the run
                if len(bb):
                    for b in np.unique(bb):
                        lo = np.searchsorted(bb, b, side="left")
                        hi = np.searchsorted(bb, b, side="right")
                        seg[(k, gg, cc, int(b))] = (lo, hi)
        core_runs.append(runs)

    T_gc = np.ceil(n_kgc / P).astype(np.int64).max(axis=0)  # [NG, NCH]
    T_gc[:, 0] = np.maximum(T_gc[:, 0], 1)

    s = Sched()
    s.T_gc = T_gc
    s.q_off = np.zeros((NG, NCH), dtype=np.int64)
    q = 0
    for g in range(NG):
        for c in range(NCH):
            s.q_off[g, c] = q
            q += T_gc[g, c]
    s.S = int(q)

    # pairs per block: [(c, j), ...]; rl column order = (b; pair index)
    s.pairs = {}            # b -> list[(c, j)]
    s.rl_off = np.zeros(NB + 1, dtype=np.int64)
    npair = 0
    for g in range(NG):
        blks = range(g * G, min(NB, (g + 1) * G))
        for b in blks:
            lst = []
            for c in range(NCH):
                jlo, jhi = None, None
                for k in range(NC):
                    sg = seg.get((k, g, c, b))
                    if sg is None:
                        continue
                    lo, hi = sg
                    a0, a1 = lo // P, (hi - 1) // P
                    jlo = a0 if jlo is None else min(jlo, a0)
                    jhi = a1 if jhi is None else max(jhi, a1)
                if jlo is None:
                    continue
                for j in range(jlo, jhi + 1):
                    lst.append((c, j))
            if not lst:
                lst = [(0, 0)]
            s.pairs[b] = lst
            s.rl_off[b] = npair
            npair += len(lst)
    s.rl_off[NB] = npair
    s.NP = npair

    # per-core idx stream + per-pair rl columns
    idxT = np.zeros((NC, P, s.S * 8), dtype=np.int16)
    rlT = np.full((NC, P, s.NP), SENT, dtype=bf16)
    for k in range(NC):
        runs = core_runs[k]
        for g in range(NG):
            for c in range(NCH):
                t = int(T_gc[g, c])
                if t == 0:
                    continue
                bb, rr, ccol = runs[(g, c)]
                n = len(bb)
                pad = t * P - n
                idx_loc = np.concatenate(
                    [ccol, np.zeros(pad, dtype=np.int64)]).astype(np.int16)
                q0 = int(s.q_off[g, c])
                qq = q0 * P + np.arange(t * P)
                idxT[k][qq % 16, qq // 16] = idx_loc
        for b in range(NB):
            g = b // G
            for i, (c, j) in enumerate(s.pairs[b]):
                bb, rr, ccol = runs[(g, c)]
                lo, hi = j * P, min((j + 1) * P, len(bb))
                if lo >= len(bb):
                    continue
                ent_b = bb[lo:hi]
                ent_r = rr[lo:hi]
                colv = np.full(P, SENT, dtype=np.float64)
                sel = ent_b == b
                colv[np.arange(hi - lo)[sel]] = ent_r[sel]
                rlT[k][:, s.rl_off[b] + i] = colv.astype(bf16)
        idxT[k] = np.tile(idxT[k][:16], (8, 1))
    s.idxT = idxT
    s.rlT = rlT
    return s


def _ap3(t, col0, nsl):
    a = t[:, col0 * P:(col0 + nsl) * P]
    return bass.AP(a.tensor, a.offset, [list(a.ap[0]), [P, nsl], [1, P]])


def _build_layer(s, layer):
    nc = bacc.Bacc("TRN2", target_bir_lowering=False, debug=False,
                   num_devices=NC, num_swdge_queues=4)
    S = s.S
    NPAIR = s.NP
    src_d = nc.dram_tensor("src", [N, D], BF16, kind="ExternalInput")
    idx_d = nc.dram_tensor("idx", [P, S * 8], mybir.dt.int16,
                           kind="ExternalInput")
    rl_d = nc.dram_tensor("rl", [P, NPAIR], BF16, kind="ExternalInput")
    m_d = nc.dram_tensor("mh", [P, NPAIR * P], BF16, kind="ExternalInput")
    iota_d = nc.dram_tensor("iota", [P, P], BF16, kind="ExternalInput")
    wa_d = nc.dram_tensor("wa", [D, D], BF16, kind="ExternalInput")
    if layer == 0:
        ba_d = nc.dram_tensor("ba", [1, D], BF16, kind="ExternalInput")
        out_d = nc.dram_tensor("h", [R, D], BF16, kind="ExternalOutput")
    else:
        ba_d = nc.dram_tensor("ba", [P, 1], mybir.dt.float32,
                              kind="ExternalInput")
        wp_d = nc.dram_tensor("wp", [D, D], BF16, kind="ExternalInput")
        bp_d = nc.dram_tensor("bp", [1, D], BF16, kind="ExternalInput")
        out_d = nc.dram_tensor("o", [R, D], mybir.dt.float32,
                               kind="ExternalOutput")

    Wmax = int(s.T_gc.max())
    Pmax = max(len(s.pairs[b]) for b in range(NB))

    with tile.TileContext(nc) as tc:
        with contextlib.ExitStack() as ctx:
            const = ctx.enter_context(tc.tile_pool(name="const", bufs=1))
            stp = [ctx.enter_context(tc.tile_pool(name=f"st{c}", bufs=3))
                   for c in range(NCH)]
            mp = ctx.enter_context(tc.tile_pool(name="mp", bufs=8))
            sp = ctx.enter_context(tc.tile_pool(name="sp", bufs=6))
            hp = ctx.enter_context(tc.tile_pool(name="hp", bufs=8))
            pa = ctx.enter_context(tc.tile_pool(name="pa", bufs=2,
                                                space="PSUM"))
            ph = ctx.enter_context(tc.tile_pool(name="ph", bufs=2,
                                                space="PSUM"))
            if layer == 1:
                po = ctx.enter_context(tc.tile_pool(name="po", bufs=2,
                                                    space="PSUM"))

            idxSB = const.tile([P, S * 8], mybir.dt.int16)
            rlSB = const.tile([P, NPAIR], BF16)
            iotaSB = const.tile([P, P], BF16)
            waSB = const.tile([D, D], BF16)
            nc.sync.dma_start(out=idxSB[:], in_=idx_d[:])
            nc.sync.dma_start(out=rlSB[:], in_=rl_d[:])
            nc.sync.dma_start(out=iotaSB[:], in_=iota_d[:])
            nc.sync.dma_start(out=waSB[:], in_=wa_d[:])
            ones1 = const.tile([1, P], BF16)
            nc.vector.memset(ones1[:], 1.0)
            if layer == 0:
                baSB = const.tile([1, D], BF16)
                nc.sync.dma_start(out=baSB[:], in_=ba_d[:])
            else:
                baSB = const.tile([P, 1], mybir.dt.float32)
                nc.sync.dma_start(out=baSB[:], in_=ba_d[:])
                wpSB = const.tile([D, D], BF16)
                bpSB = const.tile([1, D], BF16)
                nc.sync.dma_start(out=wpSB[:], in_=wp_d[:])
                nc.sync.dma_start(out=bpSB[:], in_=bp_d[:])

            nc.gpsimd.load_library(mlp_lib)

            qrr = [0]
            for g in range(NG):
                blks = range(g * G, min(NB, (g + 1) * G))
                stage = {}
                for c in range(NCH):
                    t = int(s.T_gc[g, c])
                    if t == 0:
                        continue
                    st = stp[c].tile([P, Wmax * P], BF16, tag=f"st{c}")
                    stage[c] = st
                    chlo = c * CH
                    chsz = min(CH, N - chlo)
                    done = 0
                    while done < t:
                        nn = min(t - done, MAX_SLICE_PER_GATHER)
                        q0 = int(s.q_off[g, c]) + done
                        nc.gpsimd.dma_gather(
                            out_ap=_ap3(st, done, nn),
                            in_ap=src_d[chlo:chlo + chsz, :],
                            idxs_ap=idxSB[:, q0 * 8:(q0 + nn) * 8],
                            num_idxs=nn * P, num_idxs_reg=nn * P,
                            elem_size=P, queue_num=qrr[0] % 4)
                        qrr[0] += 1
                        done += nn

                for b in blks:
                    prs = s.pairs[b]
                    npb = len(prs)
                    rows_b = min(P, R - b * P)
                    r0 = int(s.rl_off[b])

                    M = mp.tile([P, Pmax * P], BF16, tag="m")
                    if b % 2 == 0:
                        a = rlSB[:, r0:r0 + npb]
                        rl_b = bass.AP(a.tensor, a.offset,
                                       [list(a.ap[0]), [1, npb], [0, P]])
                        i0 = iotaSB[:, 0:P]
                        iota_b = bass.AP(i0.tensor, i0.offset,
                                         [list(i0.ap[0]), [0, npb], [1, P]])
                        mm = M[:, 0:npb * P]
                        m_b = bass.AP(mm.tensor, mm.offset,
                                      [list(mm.ap[0]), [P, npb], [1, P]])
                        nc.vector.tensor_tensor(out=m_b, in0=iota_b,
                                                in1=rl_b,
                                                op=mybir.AluOpType.is_equal)
                    else:
                        nc.sync.dma_start(out=M[:, 0:npb * P],
                                          in_=m_d[:, r0 * P:(r0 + npb) * P])

                    psumA = pa.tile([P, P], mybir.dt.float32, tag="pa")
                    for i, (c, j) in enumerate(prs):
                        nc.tensor.matmul(
                            psumA[:],
                            lhsT=stage[c][:, j * P:(j + 1) * P],
                            rhs=M[:, i * P:(i + 1) * P],
                            start=(i == 0), stop=(i == npb - 1))

                    sA = sp.tile([P, P], BF16, tag="sa")
                    nc.scalar.activation(sA[:], psumA[:],
                                         mybir.ActivationFunctionType.Identity)
                    if layer == 0:
                        psumH = ph.tile([P, P], mybir.dt.float32, tag="phh")
                        nc.tensor.matmul(psumH[:], lhsT=sA[:], rhs=waSB[:],
                                         start=True, stop=False)
                        nc.tensor.matmul(psumH[:], lhsT=ones1[:],
                                         rhs=baSB[:], start=False, stop=True)
                        hsb = hp.tile([P, P], BF16, tag="h")
                        nc.scalar.activation(
                            hsb[:], psumH[:],
                            mybir.ActivationFunctionType.Relu)
                        nc.sync.dma_start(
                            out=out_d[b * P:b * P + rows_b, :],
                            in_=hsb[:rows_b, :])
                    else:
                        psumZ = ph.tile([P, P], mybir.dt.float32, tag="pz")
                        nc.tensor.matmul(psumZ[:], lhsT=waSB[:], rhs=sA[:],
                                         start=True, stop=True)
                        t1 = hp.tile([P, P], BF16, tag="t1")
                        nc.scalar.activation(
                            t1[:], psumZ[:],
                            mybir.ActivationFunctionType.Relu,
                            bias=baSB[:])
                        h2T = hp.tile([P, P], BF16, tag="h2")
                        nc.vector.tensor_add(h2T[:], t1[:], sA[:])
                        psumO = po.tile([P, P], mybir.dt.float32, tag="po")
                        nc.tensor.matmul(psumO[:], lhsT=h2T[:], rhs=wpSB[:],
                                         start=True, stop=False)
                        nc.tensor.matmul(psumO[:], lhsT=ones1[:],
                                         rhs=bpSB[:], start=False, stop=True)
                        osb = hp.tile([P, P], mybir.dt.float32, tag="o")
                        nc.vector.tensor_copy(osb[:], psumO[:])
                        nc.sync.dma_start(
                            out=out_d[b * P:b * P + rows_b, :],
                            in_=osb[:rows_b, :])
    nc.compile()
    return nc


def _run(nc, in_maps):
    global LAST_EXEC_NS
    res = run_bass_kernel_spmd(nc, in_maps, core_ids=list(range(NC)),
                               trace=PROFILE)
    if PROFILE:
        LAST_EXEC_NS.append(res.exec_time_ns)
    return res.results


def kernel(x, edge_index, W0, b0, W1, b1, Wp, bp):
    global LAST_EXEC_NS
    LAST_EXEC_NS = []
    if PROFILE:
        _install_ntff_shim()
    s = _prep_edges(np.asarray(edge_index))
    iota = np.broadcast_to(
        np.arange(P, dtype=np.float32), (P, P)).astype(bf16)
    ar = np.arange(P, dtype=np.float32)
    Mh = [
        (s.rlT[k].astype(np.float32)[:, :, None] == ar[None, None, :])
        .astype(bf16).reshape(P, s.NP * P)
        for k in range(NC)
    ]

    xb = np.ascontiguousarray(np.asarray(x, np.float32)).astype(bf16)
    W0b = np.asarray(W0, np.float32).astype(bf16)
    W1b = np.asarray(W1, np.float32).astype(bf16)
    Wpb = np.asarray(Wp, np.float32).astype(bf16)
    b0b = np.asarray(b0, np.float32).reshape(1, D).astype(bf16)
    b1f = np.asarray(b1, np.float32).reshape(P, 1)
    bpb = np.asarray(bp, np.float32).reshape(1, D).astype(bf16)

    nc0 = _build_layer(s, 0)
    in0 = [{"src": xb, "idx": s.idxT[k], "rl": s.rlT[k], "iota": iota,
            "mh": Mh[k], "wa": W0b, "ba": b0b} for k in range(NC)]
    res0 = _run(nc0, in0)
    hfull = np.concatenate([res0[k]["h"] for k in range(NC)], axis=0)

    nc1 = _build_layer(s, 1)
    in1 = [{"src": hfull, "idx": s.idxT[k], "rl": s.rlT[k], "iota": iota,
            "mh": Mh[k], "wa": W1b, "ba": b1f, "wp": Wpb, "bp": bpb}
           for k in range(NC)]
    res1 = _run(nc1, in1)
    out = np.concatenate([res1[k]["o"] for k in range(NC)], axis=0)
    return out.astype(np.float32)
